# revision 1
# baseline (speedup 1.0000x reference)
"""NeRF MLP kernel for Trainium2 (Bass/Tile), 8-core data-parallel over rays.

Device layout: features on SBUF partitions, points (rays) on the free dim.
Each chunk = one sample index s for all 512 local rays (N=512 matmul moving
dim).

Key choices:
- MLP matmuls in float32r (1 cycle/row on the PE vs 4 for fp32; ~1.2e-4
  relative rounding).  Walrus requires f32r operands to be produced by a
  rounding instruction, so weights get a one-off DVE copy and activations
  are written as f32r by the ReLU that produces them.
- The positional-encoding angle path stays exact fp32 and entirely off the
  PE: angle/2pi = DF*z + AO per ray (DF, AO per-ray constants built once via
  a broadcast DMA), range reduction by the 1.5*2^23 magic-add rounding
  trick, then one ScalarE Sin per chunk.
- Compositing is restructured as w_s = T_{s-1} - T_s with
  T_s = exp(-cumsum_s sigma*dist) (the reference's 1e-10 cumprod guard only
  affects the output below ~1e-8, so it is dropped).  The cumulative sum is
  one fp32 triangular matmul in [64, B] layout at the end; per-chunk there
  is no serial dependency at all.
- sigmoid(x) = 0.5 + 0.5*tanh(x/2); tanh/exp are evaluated in the final
  phase so the chunk loop needs only {sin, relu, copy} -- a single ScalarE
  activation-table set (no per-chunk table reloads).
"""

import math
from contextlib import ExitStack

import numpy as np

import concourse.bass as bass
import concourse.mybir as mybir
import concourse.tile as tile
from concourse import bacc

F32 = mybir.dt.float32
F32R = mybir.dt.float32r
AF = mybir.ActivationFunctionType
OP = mybir.AluOpType

S = 64          # samples per ray
B_FULL = 4096   # total rays
N_CORES = 8
BL = B_FULL // N_CORES  # rays per core = 512
H = 256
NEAR, FAR = 2.0, 6.0
DELTA = (FAR - NEAR) / S
L_ENC = 5
ENC = 3 * L_ENC * 2  # 30
PI = math.pi
TWO_PI = 2.0 * math.pi
MAGIC = 12582912.0  # 1.5 * 2**23, fp32 round-to-nearest trick


def host_constants():
    """Input-independent constant tensors (same for every core)."""
    c = {}
    freqs = (2.0 ** (np.arange(L_ENC, dtype=np.float64) - 2)) * math.pi  # [L]
    fturn = np.zeros((ENC, 1), dtype=np.float32)
    phase = np.zeros((ENC, 1), dtype=np.float32)
    for cc in range(3):
        for ll in range(L_ENC):
            for tt in range(2):
                j = cc * (L_ENC * 2) + ll * 2 + tt
                fturn[j, 0] = freqs[ll] / TWO_PI
                phase[j, 0] = 0.0 if tt == 0 else 0.25  # pi/2 in turns
    c["fturn30"] = fturn
    c["phase30"] = phase

    c["cap1e10"] = np.full((1, BL), 1.0e10, dtype=np.float32)
    c["svec64"] = (NEAR + np.arange(S, dtype=np.float32)[:, None] * DELTA).astype(
        np.float32
    )
    c["ltri"] = np.triu(np.ones((S, S), dtype=np.float32))  # lhsT[t,s]=1 for t<=s
    c["ltri2"] = (np.triu(np.ones((S, S))) + np.eye(S)).astype(np.float32)
    c["ones31"] = np.ones((3, 1), dtype=np.float32)
    c["half641"] = np.full((S, 1), 0.5, dtype=np.float32)
    return c


def host_weights(inp):
    """Reshape the MLP weights into SBUF lhsT layouts."""
    w = {}

    def kstack(m):  # [256, M] -> [128, 2, M]
        return np.ascontiguousarray(m.reshape(2, 128, m.shape[1]).transpose(1, 0, 2))

    w["w0s"] = inp["w0"]                      # [30, 256]
    for i in (1, 2, 3, 5, 6):
        w[f"w{i}s"] = kstack(inp[f"w{i}"])    # [128, 2, 256]
    w["w4h"] = kstack(inp["w4"][0:H])         # [128, 2, 256]
    w["w4e"] = inp["w4"][H : H + ENC]         # [30, 256]
    w["w7f"] = kstack(inp["w7"][:, 1:129])    # [128, 2, 128]
    w["w7d"] = kstack(inp["w7"][:, 0:1])      # [128, 2, 1]
    w["w8f"] = inp["w8"][0:128]               # [128, 3]
    w["w8v"] = inp["w8"][128:131]             # [3, 3]
    for i in range(7):
        w[f"b{i}s"] = np.ascontiguousarray(inp[f"b{i}"].reshape(2, 128).T)  # [128, 2]
    w["b7f"] = np.ascontiguousarray(inp["b7"][1:129, None])  # [128, 1]
    w["b7d64"] = np.full((S, 1), inp["b7"][0], dtype=np.float32)
    for c in range(3):
        w[f"b8h64_{c}"] = np.full((S, 1), inp["b8"][c] / 2.0, dtype=np.float32)
    return w


def input_specs():
    """name -> shape for every ExternalInput of the bass program."""
    specs = {
        "xT": (6, BL),
        "off": (S, BL),
        "w0s": (30, 256),
        "w4h": (128, 2, 256),
        "w4e": (30, 256),
        "w7f": (128, 2, 128),
        "w7d": (128, 2, 1),
        "w8f": (128, 3),
        "w8v": (3, 3),
        "b7f": (128, 1),
        "b7d64": (S, 1),
        "b8h64_0": (S, 1),
        "b8h64_1": (S, 1),
        "b8h64_2": (S, 1),
        "cap1e10": (1, BL),
        "fturn30": (ENC, 1),
        "phase30": (ENC, 1),

        "svec64": (S, 1),
        "ltri": (S, S),
        "ltri2": (S, S),
        "ones31": (3, 1),
        "half641": (S, 1),
    }
    for i in (1, 2, 3, 5, 6):
        specs[f"w{i}s"] = (128, 2, 256)
    for i in range(7):
        specs[f"b{i}s"] = (128, 2)
    return specs


CONST_NAMES = (
    "w0s", "w1s", "w2s", "w3s", "w4h", "w4e", "w5s", "w6s", "w7f", "w7d",
    "w8f", "w8v", "b0s", "b1s", "b2s", "b3s", "b4s", "b5s", "b6s", "b7f",
    "b7d64", "b8h64_0", "b8h64_1", "b8h64_2", "cap1e10", "fturn30",
    "phase30", "svec64", "ltri", "ltri2", "ones31", "half641",
)

# everything that feeds the PE as float32r
MM_CONSTS = ("w0s", "w1s", "w2s", "w3s", "w4h", "w4e", "w5s", "w6s",
             "w7f", "w7d", "w8f", "w8v")


def bcast_rows(ap, reps, cols):
    """Source AP that repeats each row of `ap` `reps` times:
    [R, cols] -> [R*reps, cols] with row-major repetition (for DMA)."""
    rows = ap.shape[0]
    return bass.AP(
        tensor=ap.tensor,
        offset=ap.offset,
        ap=[[ap.ap[0][0], rows], [0, reps], [1, cols]],
    )


def build_nerf(tc, ctx, out_ap, a, repeat=1):
    """Emit the tile program.  `a` maps input name -> DRAM AP."""
    nc = tc.nc
    B = BL

    consts = ctx.enter_context(tc.tile_pool(name="consts", bufs=1))
    pre = ctx.enter_context(tc.tile_pool(name="pre", bufs=1))
    work = ctx.enter_context(tc.tile_pool(name="work", bufs=3))
    psum = ctx.enter_context(tc.tile_pool(name="psum", bufs=8, space="PSUM"))

    # ---- load constants / weights into SBUF ----
    sb = {}
    for name in CONST_NAMES:
        t = consts.tile(list(a[name].shape), F32, name=name, tag=name)
        nc.sync.dma_start(out=t, in_=a[name])
        sb[name] = t
    sr = {}
    for name in MM_CONSTS:
        t = consts.tile(list(a[name].shape), F32R, name=name + "_r", tag=name + "_r")
        nc.vector.tensor_copy(t, sb[name])
        sr[name] = t

    dt3 = pre.tile([3, B], F32, name="dt3", tag="dt3")
    nc.sync.dma_start(out=dt3, in_=a["xT"][3:6])
    off = pre.tile([S, B], F32, name="off", tag="off")
    nc.sync.dma_start(out=off, in_=a["off"])

    # per-ray encoding constants: angle/2pi = DF*z + AO  (30 rows)
    # D30/O30: d_c / o_c broadcast to rows j = c*10 + l*2 + t
    D30 = pre.tile([ENC, B], F32, name="D30", tag="D30")
    nc.sync.dma_start(out=D30, in_=bcast_rows(a["xT"][3:6], 2 * L_ENC, B))
    O30 = pre.tile([ENC, B], F32, name="O30", tag="O30")
    nc.sync.dma_start(out=O30, in_=bcast_rows(a["xT"][0:3], 2 * L_ENC, B))
    DF = pre.tile([ENC, B], F32, name="DF", tag="DF")
    nc.vector.tensor_scalar(out=DF, in0=D30, scalar1=sb["fturn30"],
                            scalar2=None, op0=OP.mult)
    AO = pre.tile([ENC, B], F32, name="AO", tag="AO")
    nc.vector.tensor_scalar(out=AO, in0=O30, scalar1=sb["fturn30"],
                            scalar2=sb["phase30"], op0=OP.mult, op1=OP.add)

    # ---- per-ray precompute ----
    # Z[s, b] = NEAR + (s + off) * DELTA
    Z = pre.tile([S, B], F32, name="Z", tag="Z")
    nc.vector.tensor_scalar(out=Z, in0=off, scalar1=DELTA, scalar2=sb["svec64"],
                            op0=OP.mult, op1=OP.add)

    # squared norm of d -> nd = |d|, inv_nd = 1/|d|  (fp32 matmul: accurate)
    sq3 = pre.tile([3, B], F32, name="sq3", tag="sq3")
    nc.vector.tensor_mul(sq3, dt3, dt3)
    ps_nd = psum.tile([128, 512], F32, name="mm", tag="mm")[0:1, :B]
    nc.tensor.matmul(ps_nd, sb["ones31"], sq3, start=True, stop=True)
    nd = pre.tile([1, B], F32, name="nd", tag="nd")
    nc.scalar.activation(out=nd, in_=ps_nd, func=AF.Sqrt)
    inv_nd = pre.tile([1, B], F32, name="inv_nd", tag="inv_nd")
    nc.vector.reciprocal(out=inv_nd, in_=nd)

    # view_dir = d / |d|   (f32r: feeds the L8 matmul)
    inv3 = pre.tile([3, B], F32, name="inv3", tag="inv3")
    nc.gpsimd.partition_broadcast(inv3, inv_nd)
    v3 = pre.tile([3, B], F32R, name="v3", tag="v3")
    nc.vector.tensor_mul(v3, dt3, inv3)

    # dists[s] = (Z[s+1]-Z[s]) * |d| for s<63, 1e10 for s=63
    nd64 = pre.tile([S, B], F32, name="nd64", tag="nd64")
    nc.gpsimd.partition_broadcast(nd64, nd)
    ZN = pre.tile([S, B], F32, name="ZN", tag="ZN")
    nc.vector.tensor_mul(ZN, Z, nd64)
    ZNs = pre.tile([S, B], F32, name="ZNs", tag="ZNs")
    nc.sync.dma_start(out=ZNs[0 : S - 1], in_=ZN[1:S])
    nc.sync.dma_start(out=ZNs[S - 1 : S], in_=a["cap1e10"])
    dists = pre.tile([S, B], F32, name="dists", tag="dists")
    nc.vector.tensor_sub(dists, ZNs, ZN)

    # phase-2 accumulators written during the chunk loop
    D64 = pre.tile([S, B], F32, name="D64", tag="D64")
    TH = [pre.tile([S, B], F32, name=f"TH{c}", tag=f"TH{c}") for c in range(3)]

    def layer(kparts, bname, h_tile, engines=("act", "vec")):
        """Dense layer with two 128-wide output chunks.
        kparts: list of (lhsT_fn(m) -> AP, rhs AP)."""
        for m, eng in enumerate(engines):
            p = psum.tile([128, 512], F32, name="mm", tag="mm")[:, :B]
            n_k = len(kparts)
            for ki, (wsl, rhs) in enumerate(kparts):
                nc.tensor.matmul(
                    p, wsl(m), rhs,
                    start=(ki == 0), stop=(ki == n_k - 1),
                )
            bias = sb[bname][:, m : m + 1]
            if eng == "act":
                nc.scalar.activation(out=h_tile[:, m, :], in_=p, func=AF.Relu,
                                     bias=bias)
            else:
                nc.vector.tensor_scalar(
                    out=h_tile[:, m, :], in0=p, scalar1=bias, scalar2=0.0,
                    op0=OP.add, op1=OP.max,
                )

    # ---- chunk loop: one sample index per chunk ----
    for s in list(range(S)) * repeat:
        # z_s broadcast to 30 rows by DMA, then the angle path on DVE (fp32)
        z30 = work.tile([ENC, B], F32, name="z30", tag="z30")
        nc.sync.dma_start(out=z30, in_=bcast_rows(Z[s : s + 1], ENC, B))
        u = work.tile([ENC, B], F32, name="u", tag="u")
        nc.vector.tensor_mul(u, DF, z30)
        u2 = work.tile([ENC, B], F32, name="u2", tag="u2")
        nc.vector.tensor_add(u2, u, AO)
        kk = work.tile([ENC, B], F32, name="kk", tag="kk")
        nc.vector.tensor_scalar(out=kk, in0=u2, scalar1=MAGIC, scalar2=MAGIC,
                                op0=OP.add, op1=OP.subtract)
        f = work.tile([ENC, B], F32, name="f", tag="f")
        nc.vector.tensor_sub(f, u2, kk)
        enc = work.tile([ENC, B], F32R, name="enc", tag="enc")
        nc.scalar.activation(out=enc, in_=f, func=AF.Sin, scale=TWO_PI)

        # ---- MLP ----
        h0 = work.tile([128, 2, B], F32R, name="h0", tag="h")
        layer([(lambda m: sr["w0s"][:, m * 128 : (m + 1) * 128], enc)], "b0s", h0)

        hp = h0
        for li in (1, 2, 3):
            h = work.tile([128, 2, B], F32R, name=f"h{li}", tag="h")
            w = sr[f"w{li}s"]
            layer(
                [
                    (lambda m, w=w: w[:, 0, m * 128 : (m + 1) * 128], hp[:, 0, :]),
                    (lambda m, w=w: w[:, 1, m * 128 : (m + 1) * 128], hp[:, 1, :]),
                ],
                f"b{li}s", h,
            )
            hp = h

        h4 = work.tile([128, 2, B], F32R, name="h4", tag="h")
        layer(
            [
                (lambda m: sr["w4h"][:, 0, m * 128 : (m + 1) * 128], hp[:, 0, :]),
                (lambda m: sr["w4h"][:, 1, m * 128 : (m + 1) * 128], hp[:, 1, :]),
                (lambda m: sr["w4e"][:, m * 128 : (m + 1) * 128], enc),
            ],
            "b4s", h4,
        )
        hp = h4
        for li in (5, 6):
            h = work.tile([128, 2, B], F32R, name=f"h{li}", tag="h")
            w = sr[f"w{li}s"]
            layer(
                [
                    (lambda m, w=w: w[:, 0, m * 128 : (m + 1) * 128], hp[:, 0, :]),
                    (lambda m, w=w: w[:, 1, m * 128 : (m + 1) * 128], hp[:, 1, :]),
                ],
                f"b{li}s", h,
            )
            hp = h

        # L7 features (fd[1:129]) -> relu -> F1 ; raw density row -> D64[s]
        p7f = psum.tile([128, 512], F32, name="mm", tag="mm")[:, :B]
        nc.tensor.matmul(p7f, sr["w7f"][:, 0, :], hp[:, 0, :],
                         start=True, stop=False)
        nc.tensor.matmul(p7f, sr["w7f"][:, 1, :], hp[:, 1, :],
                         start=False, stop=True)
        F1 = work.tile([128, B], F32R, name="F1", tag="F1")
        nc.scalar.activation(out=F1, in_=p7f, func=AF.Relu, bias=sb["b7f"])

        p7d = psum.tile([128, 512], F32, name="mm", tag="mm")[0:1, :B]
        nc.tensor.matmul(p7d, sr["w7d"][:, 0, :], hp[:, 0, :],
                         start=True, stop=False)
        nc.tensor.matmul(p7d, sr["w7d"][:, 1, :], hp[:, 1, :],
                         start=False, stop=True)
        dstage = work.tile([1, B], F32, name="dstage", tag="dstage")
        nc.scalar.activation(out=dstage, in_=p7d, func=AF.Copy)
        nc.sync.dma_start(out=D64[s : s + 1], in_=dstage)

        # L8 color pre-activation (raw rows stored; tanh happens in phase 2)
        p8 = psum.tile([128, 512], F32, name="mm", tag="mm")[0:3, :B]
        nc.tensor.matmul(p8, sr["w8f"], F1, start=True, stop=False)
        nc.tensor.matmul(p8, sr["w8v"], v3, start=False, stop=True)
        th3 = work.tile([3, B], F32, name="th3", tag="th3")
        nc.scalar.activation(out=th3, in_=p8, func=AF.Copy)
        for c in range(3):
            nc.sync.dma_start(out=TH[c][s : s + 1], in_=th3[c : c + 1])

    # ---- phase 2: compositing in [64, B] layout (fp32 matmuls) ----
    SG = pre.tile([S, B], F32, name="SG", tag="SG")
    nc.scalar.activation(out=SG, in_=D64, func=AF.Relu, bias=sb["b7d64"])
    M64 = pre.tile([S, B], F32, name="M64", tag="M64")
    nc.vector.tensor_mul(M64, SG, dists)

    # weight_s = alpha_s * prod_{t<=s}(1-alpha_t)  (inclusive cumprod)
    #          = exp(-M_s) - exp(-(M_s + m_s))
    # M_s via ltri (1s on+above diag); M_s + m_s via ltri2 (2 on diag).
    # exp(0) == 1 exactly, so fully-transparent rays give exactly zero.
    mcum = psum.tile([128, 512], F32, name="mm", tag="mm")[:S, :B]
    nc.tensor.matmul(mcum, sb["ltri"], M64, start=True, stop=True)
    vcum = psum.tile([128, 512], F32, name="mm", tag="mm")[:S, :B]
    nc.tensor.matmul(vcum, sb["ltri2"], M64, start=True, stop=True)
    T64 = pre.tile([S, B], F32, name="T64", tag="T64")
    nc.scalar.activation(out=T64, in_=mcum, func=AF.Exp, scale=-1.0)
    T64b = pre.tile([S, B], F32, name="T64b", tag="T64b")
    nc.scalar.activation(out=T64b, in_=vcum, func=AF.Exp, scale=-1.0)
    w64 = pre.tile([S, B], F32, name="w64", tag="w64")
    nc.vector.tensor_sub(w64, T64, T64b)

    # out_c = sum_s w'_s * (1 + tanh_cs)
    for c in range(3):
        THt = pre.tile([S, B], F32, name=f"THt{c}", tag=f"THt{c}")
        nc.scalar.activation(out=THt, in_=TH[c], func=AF.Tanh,
                             scale=0.5, bias=sb[f"b8h64_{c}"])
        P = pre.tile([S, B], F32, name=f"P{c}", tag=f"P{c}")
        nc.vector.tensor_mul(P, w64, THt)
        nc.vector.tensor_add(P, P, w64)
        pc = psum.tile([128, 512], F32, name="mm", tag="mm")[0:1, :B]
        nc.tensor.matmul(pc, sb["half641"], P, start=True, stop=True)
        oc = pre.tile([1, B], F32, name=f"oc{c}", tag=f"oc{c}")
        nc.scalar.activation(out=oc, in_=pc, func=AF.Copy)
        nc.sync.dma_start(out=out_ap.rearrange("b c -> c b")[c : c + 1], in_=oc)


def build_nc(repeat=1):
    nc = bacc.Bacc("TRN2", target_bir_lowering=False, debug=False)
    specs = input_specs()
    aps = {
        name: nc.dram_tensor(name, list(shape), F32, kind="ExternalInput").ap()
        for name, shape in specs.items()
    }
    out = nc.dram_tensor("out", [BL, 3], F32, kind="ExternalOutput").ap()
    with tile.TileContext(nc) as tc, ExitStack() as ctx:
        build_nerf(tc, ctx, out, aps, repeat=repeat)
    nc.compile()
    return nc


def make_in_maps(inputs):
    consts = host_constants()
    wts = host_weights(inputs)
    shared = {**consts, **wts}
    shared = {k: np.ascontiguousarray(v, dtype=np.float32) for k, v in shared.items()}
    in_maps = []
    for core in range(N_CORES):
        sl = slice(core * BL, (core + 1) * BL)
        m = dict(shared)
        m["xT"] = np.ascontiguousarray(np.asarray(inputs["x"])[sl].T, dtype=np.float32)
        m["off"] = np.ascontiguousarray(
            np.asarray(inputs["offsets"])[:, sl], dtype=np.float32
        )
        in_maps.append(m)
    return in_maps


def kernel(**inputs):
    from concourse.bass_utils import run_bass_kernel_spmd

    nc = build_nc()
    in_maps = make_in_maps(inputs)
    res = run_bass_kernel_spmd(nc, in_maps, core_ids=list(range(N_CORES)))
    out = np.concatenate([r["out"] for r in res.results], axis=0)
    return out.astype(np.float32)



# revision 2
# speedup vs baseline: 1.1083x; 1.1083x over previous
"""NeRF MLP kernel for Trainium2 (Bass/Tile), 8-core data-parallel over rays.

Device layout: features on SBUF partitions, points (rays) on the free dim.
Each chunk = one sample index s for all 512 local rays (N=512 matmul moving
dim).

Key choices:
- MLP matmuls in float32r (1 cycle/row on the PE vs 4 for fp32; ~1.2e-4
  relative rounding).  Walrus requires f32r operands to be produced by a
  rounding instruction, so weights get a one-off DVE copy and activations
  are written as f32r by the ReLU that produces them.
- The positional-encoding angle path stays exact fp32 and entirely off the
  PE: angle/2pi = DF*z + AO per ray (DF, AO per-ray constants built once via
  a broadcast DMA), range reduction by the 1.5*2^23 magic-add rounding
  trick, then one ScalarE Sin per chunk.
- Compositing is restructured as w_s = T_{s-1} - T_s with
  T_s = exp(-cumsum_s sigma*dist) (the reference's 1e-10 cumprod guard only
  affects the output below ~1e-8, so it is dropped).  The cumulative sum is
  one fp32 triangular matmul in [64, B] layout at the end; per-chunk there
  is no serial dependency at all.
- sigmoid(x) = 0.5 + 0.5*tanh(x/2); tanh/exp are evaluated in the final
  phase so the chunk loop needs only {sin, relu, copy} -- a single ScalarE
  activation-table set (no per-chunk table reloads).
"""

import math
from contextlib import ExitStack

import numpy as np

import concourse.bass as bass
import concourse.mybir as mybir
import concourse.tile as tile
from concourse import bacc

F32 = mybir.dt.float32
F32R = mybir.dt.float32r
BF16 = mybir.dt.bfloat16
AF = mybir.ActivationFunctionType
OP = mybir.AluOpType

S = 64          # samples per ray
B_FULL = 4096   # total rays
N_CORES = 8
BL = B_FULL // N_CORES  # rays per core = 512
H = 256
NEAR, FAR = 2.0, 6.0
DELTA = (FAR - NEAR) / S
L_ENC = 5
ENC = 3 * L_ENC * 2  # 30
PI = math.pi
TWO_PI = 2.0 * math.pi
MAGIC = 12582912.0  # 1.5 * 2**23, fp32 round-to-nearest trick


def host_constants():
    """Input-independent constant tensors (same for every core)."""
    c = {}
    freqs = (2.0 ** (np.arange(L_ENC, dtype=np.float64) - 2)) * math.pi  # [L]
    fturn = np.zeros((ENC, 1), dtype=np.float32)
    phase = np.zeros((ENC, 1), dtype=np.float32)
    for cc in range(3):
        for ll in range(L_ENC):
            for tt in range(2):
                j = cc * (L_ENC * 2) + ll * 2 + tt
                fturn[j, 0] = freqs[ll] / TWO_PI
                phase[j, 0] = 0.0 if tt == 0 else 0.25  # pi/2 in turns
    c["fturn30"] = fturn
    c["phase30"] = phase

    c["cap1e10"] = np.full((1, BL), 1.0e10, dtype=np.float32)
    c["svec64"] = (NEAR + np.arange(S, dtype=np.float32)[:, None] * DELTA).astype(
        np.float32
    )
    c["ltri"] = np.triu(np.ones((S, S), dtype=np.float32))  # lhsT[t,s]=1 for t<=s
    c["ltri2"] = (np.triu(np.ones((S, S))) + np.eye(S)).astype(np.float32)
    c["ones31"] = np.ones((3, 1), dtype=np.float32)
    c["half641"] = np.full((S, 1), 0.5, dtype=np.float32)
    return c


def host_weights(inp):
    """Reshape the MLP weights into SBUF lhsT layouts."""
    w = {}

    def kstack(m):  # [256, M] -> [128, 2, M]
        return np.ascontiguousarray(m.reshape(2, 128, m.shape[1]).transpose(1, 0, 2))

    w["w0s"] = inp["w0"]                      # [30, 256]
    for i in (1, 2, 3, 5, 6):
        w[f"w{i}s"] = kstack(inp[f"w{i}"])    # [128, 2, 256]
    w["w4h"] = kstack(inp["w4"][0:H])         # [128, 2, 256]
    w["w4e"] = inp["w4"][H : H + ENC]         # [30, 256]
    w["w7f"] = kstack(inp["w7"][:, 1:129])    # [128, 2, 128]
    w["w7d"] = kstack(inp["w7"][:, 0:1])      # [128, 2, 1]
    w["w8f"] = inp["w8"][0:128]               # [128, 3]
    w["w8v"] = inp["w8"][128:131]             # [3, 3]
    for i in range(7):
        w[f"b{i}s"] = np.ascontiguousarray(inp[f"b{i}"].reshape(2, 128).T)  # [128, 2]
    w["b7f"] = np.ascontiguousarray(inp["b7"][1:129, None])  # [128, 1]
    w["b7d64"] = np.full((S, 1), inp["b7"][0], dtype=np.float32)
    for c in range(3):
        w[f"b8h64_{c}"] = np.full((S, 1), inp["b8"][c] / 2.0, dtype=np.float32)
    return w


def input_specs():
    """name -> shape for every ExternalInput of the bass program."""
    specs = {
        "xT": (6, BL),
        "off": (S, BL),
        "w0s": (30, 256),
        "w4h": (128, 2, 256),
        "w4e": (30, 256),
        "w7f": (128, 2, 128),
        "w7d": (128, 2, 1),
        "w8f": (128, 3),
        "w8v": (3, 3),
        "b7f": (128, 1),
        "b7d64": (S, 1),
        "b8h64_0": (S, 1),
        "b8h64_1": (S, 1),
        "b8h64_2": (S, 1),
        "cap1e10": (1, BL),
        "fturn30": (ENC, 1),
        "phase30": (ENC, 1),

        "svec64": (S, 1),
        "ltri": (S, S),
        "ltri2": (S, S),
        "ones31": (3, 1),
        "half641": (S, 1),
    }
    for i in (1, 2, 3, 5, 6):
        specs[f"w{i}s"] = (128, 2, 256)
    for i in range(7):
        specs[f"b{i}s"] = (128, 2)
    return specs


CONST_NAMES = (
    "w0s", "w1s", "w2s", "w3s", "w4h", "w4e", "w5s", "w6s", "w7f", "w7d",
    "w8f", "w8v", "b0s", "b1s", "b2s", "b3s", "b4s", "b5s", "b6s", "b7f",
    "b7d64", "b8h64_0", "b8h64_1", "b8h64_2", "cap1e10", "fturn30",
    "phase30", "svec64", "ltri", "ltri2", "ones31", "half641",
)

# everything that feeds the PE as float32r
MM_CONSTS = ("w0s", "w1s", "w2s", "w3s", "w4h", "w4e", "w5s", "w6s",
             "w7f", "w7d", "w8f", "w8v")


def bcast_rows(ap, reps, cols):
    """Source AP that repeats each row of `ap` `reps` times:
    [R, cols] -> [R*reps, cols] with row-major repetition (for DMA)."""
    rows = ap.shape[0]
    return bass.AP(
        tensor=ap.tensor,
        offset=ap.offset,
        ap=[[ap.ap[0][0], rows], [0, reps], [1, cols]],
    )


def build_nerf(tc, ctx, out_ap, a, repeat=1):
    """Emit the tile program.  `a` maps input name -> DRAM AP."""
    nc = tc.nc
    B = BL

    consts = ctx.enter_context(tc.tile_pool(name="consts", bufs=1))
    pre = ctx.enter_context(tc.tile_pool(name="pre", bufs=1))
    work = ctx.enter_context(tc.tile_pool(name="work", bufs=3))
    psum = ctx.enter_context(tc.tile_pool(name="psum", bufs=8, space="PSUM"))

    # ---- load constants / weights into SBUF ----
    sb = {}
    for name in CONST_NAMES:
        t = consts.tile(list(a[name].shape), F32, name=name, tag=name)
        nc.sync.dma_start(out=t, in_=a[name])
        sb[name] = t
    sr = {}
    for name in MM_CONSTS:
        t = consts.tile(list(a[name].shape), BF16, name=name + "_r", tag=name + "_r")
        nc.vector.tensor_copy(t, sb[name])
        sr[name] = t

    dt3 = pre.tile([3, B], F32, name="dt3", tag="dt3")
    nc.sync.dma_start(out=dt3, in_=a["xT"][3:6])
    off = pre.tile([S, B], F32, name="off", tag="off")
    nc.sync.dma_start(out=off, in_=a["off"])

    # per-ray encoding constants: angle/2pi = DF*z + AO  (30 rows)
    # D30/O30: d_c / o_c broadcast to rows j = c*10 + l*2 + t
    D30 = pre.tile([ENC, B], F32, name="D30", tag="D30")
    nc.sync.dma_start(out=D30, in_=bcast_rows(a["xT"][3:6], 2 * L_ENC, B))
    O30 = pre.tile([ENC, B], F32, name="O30", tag="O30")
    nc.sync.dma_start(out=O30, in_=bcast_rows(a["xT"][0:3], 2 * L_ENC, B))
    DF = pre.tile([ENC, B], F32, name="DF", tag="DF")
    nc.vector.tensor_scalar(out=DF, in0=D30, scalar1=sb["fturn30"],
                            scalar2=None, op0=OP.mult)
    AO = pre.tile([ENC, B], F32, name="AO", tag="AO")
    nc.vector.tensor_scalar(out=AO, in0=O30, scalar1=sb["fturn30"],
                            scalar2=sb["phase30"], op0=OP.mult, op1=OP.add)

    # ---- per-ray precompute ----
    # Z[s, b] = NEAR + (s + off) * DELTA
    Z = pre.tile([S, B], F32, name="Z", tag="Z")
    nc.vector.tensor_scalar(out=Z, in0=off, scalar1=DELTA, scalar2=sb["svec64"],
                            op0=OP.mult, op1=OP.add)

    # squared norm of d -> nd = |d|, inv_nd = 1/|d|  (fp32 matmul: accurate)
    sq3 = pre.tile([3, B], F32, name="sq3", tag="sq3")
    nc.vector.tensor_mul(sq3, dt3, dt3)
    ps_nd = psum.tile([128, 512], F32, name="mm", tag="mm")[0:1, :B]
    nc.tensor.matmul(ps_nd, sb["ones31"], sq3, start=True, stop=True)
    nd = pre.tile([1, B], F32, name="nd", tag="nd")
    nc.scalar.activation(out=nd, in_=ps_nd, func=AF.Sqrt)
    inv_nd = pre.tile([1, B], F32, name="inv_nd", tag="inv_nd")
    nc.vector.reciprocal(out=inv_nd, in_=nd)

    # view_dir = d / |d|   (f32r: feeds the L8 matmul)
    inv3 = pre.tile([3, B], F32, name="inv3", tag="inv3")
    nc.gpsimd.partition_broadcast(inv3, inv_nd)
    v3 = pre.tile([3, B], BF16, name="v3", tag="v3")
    nc.vector.tensor_mul(v3, dt3, inv3)

    # dists[s] = (Z[s+1]-Z[s]) * |d| for s<63, 1e10 for s=63
    nd64 = pre.tile([S, B], F32, name="nd64", tag="nd64")
    nc.gpsimd.partition_broadcast(nd64, nd)
    ZN = pre.tile([S, B], F32, name="ZN", tag="ZN")
    nc.vector.tensor_mul(ZN, Z, nd64)
    ZNs = pre.tile([S, B], F32, name="ZNs", tag="ZNs")
    nc.sync.dma_start(out=ZNs[0 : S - 1], in_=ZN[1:S])
    nc.sync.dma_start(out=ZNs[S - 1 : S], in_=a["cap1e10"])
    dists = pre.tile([S, B], F32, name="dists", tag="dists")
    nc.vector.tensor_sub(dists, ZNs, ZN)

    # phase-2 accumulators written during the chunk loop
    D64 = pre.tile([S, B], F32, name="D64", tag="D64")
    TH = [pre.tile([S, B], F32, name=f"TH{c}", tag=f"TH{c}") for c in range(3)]

    def layer(kparts, bname, h_tile, engines=("act", "vec")):
        """Dense layer with two 128-wide output chunks.
        kparts: list of (lhsT_fn(m) -> AP, rhs AP)."""
        for m, eng in enumerate(engines):
            p = psum.tile([128, 512], F32, name="mm", tag="mm")[:, :B]
            n_k = len(kparts)
            for ki, (wsl, rhs) in enumerate(kparts):
                nc.tensor.matmul(
                    p, wsl(m), rhs,
                    start=(ki == 0), stop=(ki == n_k - 1),
                )
            bias = sb[bname][:, m : m + 1]
            if eng == "act":
                nc.scalar.activation(out=h_tile[:, m, :], in_=p, func=AF.Relu,
                                     bias=bias)
            else:
                nc.vector.tensor_scalar(
                    out=h_tile[:, m, :], in0=p, scalar1=bias, scalar2=0.0,
                    op0=OP.add, op1=OP.max,
                )

    # ---- chunk loop: one sample index per chunk ----
    for s in list(range(S)) * repeat:
        # z_s broadcast to 30 rows by DMA, then the angle path on DVE (fp32)
        z30 = work.tile([ENC, B], F32, name="z30", tag="z30")
        nc.sync.dma_start(out=z30, in_=bcast_rows(Z[s : s + 1], ENC, B))
        u = work.tile([ENC, B], F32, name="u", tag="u")
        nc.vector.tensor_mul(u, DF, z30)
        u2 = work.tile([ENC, B], F32, name="u2", tag="u2")
        nc.vector.tensor_add(u2, u, AO)
        kk = work.tile([ENC, B], F32, name="kk", tag="kk")
        nc.vector.tensor_scalar(out=kk, in0=u2, scalar1=MAGIC, scalar2=MAGIC,
                                op0=OP.add, op1=OP.subtract)
        f = work.tile([ENC, B], F32, name="f", tag="f")
        nc.vector.tensor_sub(f, u2, kk)
        enc = work.tile([ENC, B], BF16, name="enc", tag="enc")
        nc.scalar.activation(out=enc, in_=f, func=AF.Sin, scale=TWO_PI)

        # ---- MLP ----
        h0 = work.tile([128, 2, B], BF16, name="h0", tag="h")
        layer([(lambda m: sr["w0s"][:, m * 128 : (m + 1) * 128], enc)], "b0s", h0)

        hp = h0
        for li in (1, 2, 3):
            h = work.tile([128, 2, B], BF16, name=f"h{li}", tag="h")
            w = sr[f"w{li}s"]
            layer(
                [
                    (lambda m, w=w: w[:, 0, m * 128 : (m + 1) * 128], hp[:, 0, :]),
                    (lambda m, w=w: w[:, 1, m * 128 : (m + 1) * 128], hp[:, 1, :]),
                ],
                f"b{li}s", h,
            )
            hp = h

        h4 = work.tile([128, 2, B], BF16, name="h4", tag="h")
        layer(
            [
                (lambda m: sr["w4h"][:, 0, m * 128 : (m + 1) * 128], hp[:, 0, :]),
                (lambda m: sr["w4h"][:, 1, m * 128 : (m + 1) * 128], hp[:, 1, :]),
                (lambda m: sr["w4e"][:, m * 128 : (m + 1) * 128], enc),
            ],
            "b4s", h4,
        )
        hp = h4
        for li in (5, 6):
            h = work.tile([128, 2, B], BF16, name=f"h{li}", tag="h")
            w = sr[f"w{li}s"]
            layer(
                [
                    (lambda m, w=w: w[:, 0, m * 128 : (m + 1) * 128], hp[:, 0, :]),
                    (lambda m, w=w: w[:, 1, m * 128 : (m + 1) * 128], hp[:, 1, :]),
                ],
                f"b{li}s", h,
            )
            hp = h

        # L7 features (fd[1:129]) -> relu -> F1 ; raw density row -> D64[s]
        p7f = psum.tile([128, 512], F32, name="mm", tag="mm")[:, :B]
        nc.tensor.matmul(p7f, sr["w7f"][:, 0, :], hp[:, 0, :],
                         start=True, stop=False)
        nc.tensor.matmul(p7f, sr["w7f"][:, 1, :], hp[:, 1, :],
                         start=False, stop=True)
        F1 = work.tile([128, B], BF16, name="F1", tag="F1")
        nc.scalar.activation(out=F1, in_=p7f, func=AF.Relu, bias=sb["b7f"])

        p7d = psum.tile([128, 512], F32, name="mm", tag="mm")[0:1, :B]
        nc.tensor.matmul(p7d, sr["w7d"][:, 0, :], hp[:, 0, :],
                         start=True, stop=False)
        nc.tensor.matmul(p7d, sr["w7d"][:, 1, :], hp[:, 1, :],
                         start=False, stop=True)
        dstage = work.tile([1, B], F32, name="dstage", tag="dstage")
        nc.scalar.activation(out=dstage, in_=p7d, func=AF.Copy)
        nc.sync.dma_start(out=D64[s : s + 1], in_=dstage)

        # L8 color pre-activation (raw rows stored; tanh happens in phase 2)
        p8 = psum.tile([128, 512], F32, name="mm", tag="mm")[0:3, :B]
        nc.tensor.matmul(p8, sr["w8f"], F1, start=True, stop=False)
        nc.tensor.matmul(p8, sr["w8v"], v3, start=False, stop=True)
        th3 = work.tile([3, B], F32, name="th3", tag="th3")
        nc.scalar.activation(out=th3, in_=p8, func=AF.Copy)
        for c in range(3):
            nc.sync.dma_start(out=TH[c][s : s + 1], in_=th3[c : c + 1])

    # ---- phase 2: compositing in [64, B] layout (fp32 matmuls) ----
    SG = pre.tile([S, B], F32, name="SG", tag="SG")
    nc.scalar.activation(out=SG, in_=D64, func=AF.Relu, bias=sb["b7d64"])
    M64 = pre.tile([S, B], F32, name="M64", tag="M64")
    nc.vector.tensor_mul(M64, SG, dists)

    # weight_s = alpha_s * prod_{t<=s}(1-alpha_t)  (inclusive cumprod)
    #          = exp(-M_s) - exp(-(M_s + m_s))
    # M_s via ltri (1s on+above diag); M_s + m_s via ltri2 (2 on diag).
    # exp(0) == 1 exactly, so fully-transparent rays give exactly zero.
    mcum = psum.tile([128, 512], F32, name="mm", tag="mm")[:S, :B]
    nc.tensor.matmul(mcum, sb["ltri"], M64, start=True, stop=True)
    vcum = psum.tile([128, 512], F32, name="mm", tag="mm")[:S, :B]
    nc.tensor.matmul(vcum, sb["ltri2"], M64, start=True, stop=True)
    T64 = pre.tile([S, B], F32, name="T64", tag="T64")
    nc.scalar.activation(out=T64, in_=mcum, func=AF.Exp, scale=-1.0)
    T64b = pre.tile([S, B], F32, name="T64b", tag="T64b")
    nc.scalar.activation(out=T64b, in_=vcum, func=AF.Exp, scale=-1.0)
    w64 = pre.tile([S, B], F32, name="w64", tag="w64")
    nc.vector.tensor_sub(w64, T64, T64b)

    # out_c = sum_s w'_s * (1 + tanh_cs)
    for c in range(3):
        THt = pre.tile([S, B], F32, name=f"THt{c}", tag=f"THt{c}")
        nc.scalar.activation(out=THt, in_=TH[c], func=AF.Tanh,
                             scale=0.5, bias=sb[f"b8h64_{c}"])
        P = pre.tile([S, B], F32, name=f"P{c}", tag=f"P{c}")
        nc.vector.tensor_mul(P, w64, THt)
        nc.vector.tensor_add(P, P, w64)
        pc = psum.tile([128, 512], F32, name="mm", tag="mm")[0:1, :B]
        nc.tensor.matmul(pc, sb["half641"], P, start=True, stop=True)
        oc = pre.tile([1, B], F32, name=f"oc{c}", tag=f"oc{c}")
        nc.scalar.activation(out=oc, in_=pc, func=AF.Copy)
        nc.sync.dma_start(out=out_ap.rearrange("b c -> c b")[c : c + 1], in_=oc)


def build_nc(repeat=1):
    nc = bacc.Bacc("TRN2", target_bir_lowering=False, debug=False)
    specs = input_specs()
    aps = {
        name: nc.dram_tensor(name, list(shape), F32, kind="ExternalInput").ap()
        for name, shape in specs.items()
    }
    out = nc.dram_tensor("out", [BL, 3], F32, kind="ExternalOutput").ap()
    with tile.TileContext(nc) as tc, ExitStack() as ctx:
        build_nerf(tc, ctx, out, aps, repeat=repeat)
    nc.compile()
    return nc


def make_in_maps(inputs):
    consts = host_constants()
    wts = host_weights(inputs)
    shared = {**consts, **wts}
    shared = {k: np.ascontiguousarray(v, dtype=np.float32) for k, v in shared.items()}
    in_maps = []
    for core in range(N_CORES):
        sl = slice(core * BL, (core + 1) * BL)
        m = dict(shared)
        m["xT"] = np.ascontiguousarray(np.asarray(inputs["x"])[sl].T, dtype=np.float32)
        m["off"] = np.ascontiguousarray(
            np.asarray(inputs["offsets"])[:, sl], dtype=np.float32
        )
        in_maps.append(m)
    return in_maps


def kernel(**inputs):
    from concourse.bass_utils import run_bass_kernel_spmd

    nc = build_nc()
    in_maps = make_in_maps(inputs)
    res = run_bass_kernel_spmd(nc, in_maps, core_ids=list(range(N_CORES)))
    out = np.concatenate([r["out"] for r in res.results], axis=0)
    return out.astype(np.float32)

